# revision 44
# baseline (speedup 1.0000x reference)
"""Trainium2 Bass kernel for CustomGNNvA (2-layer GATv2 + BN/ELU + MLP head).

Self-contained: hardcodes all shapes. Edge-parallel sharding by destination
node range across 8 NeuronCores; per-core local softmax-aggregation via
one-hot matmuls; cross-core collectives for BN stats (AllReduce) and the
per-layer xl gather table (AllGather of per-core slices).

v2 vs baseline:
 - node transform split across cores (1/8 each) + AllGather of the xl pair
   table (replaces per-core full-N transform + activation AllGather)
 - one-hot dst-select matrix built on-chip (PE transpose of the DVE-built
   one-hot) instead of streaming a 116-row one-hot table from HBM
 - LeakyReLU as a single DVE scalar_tensor_tensor max(0.2x, x)
 - payload multiply on DVE (was GPSIMD), parity select as one DVE select
 - NB=128 dst nodes per block, exp held in bf16 inside the payload tile
"""
import sys

sys.path.insert(0, "/opt/trn_rl_repo")

import math
from contextlib import ExitStack
from dataclasses import dataclass

import numpy as np
import ml_dtypes

from concourse import bass, mybir, tile, bacc
from concourse.bass_utils import run_bass_kernel_spmd
from concourse.masks import make_identity

BF16 = ml_dtypes.bfloat16
P = 128


@dataclass
class Cfg:
    N: int = 50000
    E: int = 1600000
    D_IN: int = 128
    H: int = 4
    C: int = 16
    HC: int = 64
    HS2: int = 64
    EDGE_DIM: int = 4
    EPS: float = 1e-5
    CORES: int = 8
    NB: int = 124          # dst nodes per block (one-hot rows; +4 eattr rows)

    @property
    def NL(self):  # nodes per core
        return self.N // self.CORES

    @property
    def NBLK(self):  # dst blocks per core
        return math.ceil(self.NL / self.NB)

    @property
    def NLOC(self):  # padded local node count
        return self.NBLK * self.NB

    @property
    def NLPAD(self):  # local nodes padded to 128-node table tiles
        return math.ceil(self.NLOC / 128) * 128

    @property
    def PAIRS(self):  # pair rows per core (padded to table tiles)
        return self.NLPAD // 2


FULL = Cfg()


# ---------------------------------------------------------------- CPU prep

def prep(cfg: Cfg, data_x, data_edge_index, data_edge_attr, weights: dict):
    """Shard + reorder edges; build all per-core input arrays (layout only)."""
    src = np.asarray(data_edge_index[0]).astype(np.int64)
    dst = np.asarray(data_edge_index[1]).astype(np.int64)
    eattr = np.asarray(data_edge_attr, np.float32)
    NL, NB, NBLK, CORES = cfg.NL, cfg.NB, cfg.NBLK, cfg.CORES

    core = dst // NL
    dstloc = dst % NL
    blk = dstloc // NB
    dstin = dstloc % NB
    gkey = core * NBLK + blk
    counts = np.bincount(gkey, minlength=CORES * NBLK)
    T_B = max(1, int(math.ceil(counts.max() / P)))
    ET = T_B * P                       # padded edges per block
    TT = NBLK * T_B                    # tiles per core
    TE = NBLK * ET                     # padded edges per core

    # position of each edge in its core's padded array
    order = np.argsort(gkey, kind="stable")
    within = np.arange(cfg.E) - np.concatenate([[0], np.cumsum(counts)])[gkey[order]]
    pos = np.empty(cfg.E, np.int64)
    pos[order] = (blk[order] * ET) + within

    # global pair row of a source node (pair table is per-core slices of
    # PAIRS rows concatenated)
    pairrow = (src // NL) * cfg.PAIRS + (src % NL) // 2

    in_maps = [dict() for _ in range(CORES)]
    for c in range(CORES):
        sel = core == c
        p_c = pos[sel]
        row_e = np.zeros(TE, np.int64)
        par_e = np.zeros(TE, np.int64)
        dstin_e = np.zeros(TE, np.int64)
        valid_e = np.zeros(TE, np.float32)
        eattr_e = np.zeros((TE, cfg.EDGE_DIM), np.float32)
        row_e[p_c] = pairrow[sel]
        par_e[p_c] = src[sel] & 1
        dstin_e[p_c] = dstin[sel]
        valid_e[p_c] = 1.0
        eattr_e[p_c] = eattr[sel]

        def wrap(x):  # [TE] -> [128, TT]; edge j=(t*128+p) at [p, t]
            return np.ascontiguousarray(x.reshape(TT, P).T)

        m = in_maps[c]
        m["dstloc_w"] = wrap(dstin_e.astype(np.float32)).astype(BF16)
        m["parity_w"] = wrap(par_e.astype(np.float32)).astype(np.uint8)
        m["valid_w"] = wrap(valid_e).astype(BF16)
        # gather indices: per chunk, wrapped-16 and replicated to 128 partitions
        idx = row_e.astype(np.int16).reshape(NBLK, ET)
        iw = np.zeros((NBLK, P, ET // 16), np.int16)
        for g in range(8):
            iw[:, g * 16:(g + 1) * 16, :] = idx.reshape(NBLK, ET // 16, 16).transpose(0, 2, 1)
        m["srcpair_w"] = np.ascontiguousarray(iw.transpose(1, 0, 2).reshape(P, NBLK * (ET // 16)))
        # edge-major eattr, wrapped like dstloc: [128, TT, 4]
        m["eattr_em"] = np.ascontiguousarray(
            eattr_e.reshape(TT, P, cfg.EDGE_DIM).transpose(1, 0, 2)
            .reshape(P, TT * cfg.EDGE_DIM)).astype(BF16)

        xl = np.zeros((cfg.D_IN, cfg.NLPAD), np.float32)
        xl[:, :NL] = np.asarray(data_x, np.float32).T[:, c * NL:(c + 1) * NL]
        m["xT_loc"] = xl
        for k, v in weights.items():
            m[k] = v
    return in_maps, T_B


def prep_weights(cfg: Cfg, inp: dict):
    w = {}
    for l in range(2):
        w[f"Wl{l}"] = np.asarray(inp[f"Wl{l}"], np.float32)
        w[f"Wr{l}"] = np.asarray(inp[f"Wr{l}"], np.float32)
        w[f"We{l}"] = np.asarray(inp[f"We{l}"], np.float32).astype(BF16)
        w[f"We_rows{l}"] = np.tile(w[f"We{l}"], (1, cfg.NBLK))
        w[f"bl{l}"] = np.asarray(inp[f"bl{l}"], np.float32).reshape(-1, 1)
        w[f"br{l}"] = np.asarray(inp[f"br{l}"], np.float32).reshape(-1, 1)
        w[f"att{l}"] = np.asarray(inp[f"att{l}"], np.float32).reshape(1, -1).astype(BF16)
        w[f"g{l}"] = np.asarray(inp[f"g{l}"], np.float32).reshape(-1, 1)
        w[f"be{l}"] = np.asarray(inp[f"be{l}"], np.float32).reshape(-1, 1)
    w["W1"] = np.asarray(inp["W1"], np.float32)
    w["W2"] = np.asarray(inp["W2"], np.float32).astype(BF16)
    w["gf"] = np.asarray(inp["gf"], np.float32).reshape(-1, 1)
    w["bf"] = np.asarray(inp["bf"], np.float32).reshape(-1, 1)
    w["b2"] = np.asarray(inp["b2"], np.float32).reshape(1, 1)
    return w


# ---------------------------------------------------------------- builder

def build(cfg: Cfg, T_B: int, ablate: frozenset = frozenset()):
    nq = 4 if "q4" in ablate else (2 if "q2" in ablate else 1)
    nc = bacc.Bacc(None, target_bir_lowering=False, debug=False,
                   num_devices=cfg.CORES, num_swdge_queues=nq)
    f32, b16, i16 = mybir.dt.float32, mybir.dt.bfloat16, mybir.dt.int16
    AF = mybir.ActivationFunctionType
    OP = mybir.AluOpType
    NB, NBLK, NL, HC, H, C = cfg.NB, cfg.NBLK, cfg.NL, cfg.HC, cfg.H, cfg.C
    ET = T_B * P
    TT = NBLK * T_B
    NLP = cfg.NLPAD
    PAIRS = cfg.PAIRS
    PAY = HC + H  # 68

    # ---- dram parameters
    dp = {}
    def param(name, shape, dt):
        dp[name] = nc.declare_dram_parameter(name, list(shape), dt, isOutput=False)
        return dp[name]

    RANKS = cfg.CORES * PAIRS // P
    NLPAD = cfg.NLPAD
    if "fewparams" in ablate:
        param("xT_loc", [cfg.D_IN, NLPAD], f32)
        out_p = nc.declare_dram_parameter("out", [1, NL], f32, isOutput=True)
        with tile.TileContext(nc) as tc, ExitStack() as ctx:
            pool = ctx.enter_context(tc.tile_pool(name="fp", bufs=1))
            t = pool.tile([1, NL], f32, name="t")
            nc.sync.dma_start(out=t[:], in_=dp["xT_loc"][0:1, :NL])
            nc.sync.dma_start(out=out_p[:], in_=t[:])
        nc.compile()
        return nc
    param("dstloc_w", [P, TT], b16)
    param("parity_w", [P, TT], mybir.dt.uint8)
    param("valid_w", [P, TT], b16)
    param("srcpair_w", [P, TT * 8], i16)
    param("eattr_em", [P, TT * cfg.EDGE_DIM], b16)
    param("xT_loc", [cfg.D_IN, NLPAD], f32)
    for l in range(2):
        din = cfg.D_IN if l == 0 else HC
        param(f"Wl{l}", [din, HC], f32)
        param(f"Wr{l}", [din, HC], f32)
        param(f"We{l}", [cfg.EDGE_DIM, HC], b16)
        param(f"We_rows{l}", [cfg.EDGE_DIM, NBLK * HC], b16)
        param(f"bl{l}", [HC, 1], f32)
        param(f"br{l}", [HC, 1], f32)
        param(f"att{l}", [1, HC], b16)
        param(f"g{l}", [HC, 1], f32)
        param(f"be{l}", [HC, 1], f32)
    param("W1", [HC, cfg.HS2], f32)
    param("W2", [cfg.HS2, 1], b16)
    param("gf", [cfg.HS2, 1], f32)
    param("bf", [cfg.HS2, 1], f32)
    param("b2", [1, 1], f32)
    out_p = nc.declare_dram_parameter("out", [1, NL], f32, isOutput=True)
    dbg_p = None
    if "debug" in ablate:
        dbg_p = nc.declare_dram_parameter("dbg", [cfg.HC, cfg.NLPAD], f32,
                                          isOutput=True)

    with tile.TileContext(nc) as tc, ExitStack() as ctx:
        consts = ctx.enter_context(tc.tile_pool(name="consts", bufs=1))
        resident = ctx.enter_context(tc.tile_pool(name="resident", bufs=1))
        dram = ctx.enter_context(tc.tile_pool(name="dram", bufs=1, space="DRAM"))

        # ---- constants in SBUF
        ident = consts.tile([P, P], b16, name="ident")
        make_identity(nc, ident[:])
        identF = consts.tile([P, P], f32, name="identF")
        make_identity(nc, identF[:])
        iota32 = consts.tile([P, P], mybir.dt.int32, name="iota32")
        nc.gpsimd.iota(iota32[:], pattern=[[1, P]], base=0, channel_multiplier=0)
        iotaB = consts.tile([P, P], b16, name="iotaB")
        nc.vector.tensor_copy(iotaB[:], iota32[:])
        c_tiny = consts.tile([P, 1], f32, name="c_tiny")
        nc.vector.memset(c_tiny[:], 1e-16)
        c_eps = consts.tile([P, 1], f32, name="c_eps")
        nc.vector.memset(c_eps[:], cfg.EPS)

        def load_sb(pool, name, shape, dt, bcast_p=None):
            t = pool.tile(list(shape), dt, name=f"sb_{name}")
            srcap = dp[name][:]
            if bcast_p is not None:
                srcap = bass.AP(tensor=srcap.tensor, offset=srcap.offset,
                                ap=[[0, bcast_p]] + srcap.ap[1:])
            nc.sync.dma_start(out=t[:], in_=srcap)
            return t

        wsb = {}
        for l in range(2):
            din = cfg.D_IN if l == 0 else HC
            wsb[f"Wl{l}"] = load_sb(consts, f"Wl{l}", [din, HC], f32)
            wsb[f"Wr{l}"] = load_sb(consts, f"Wr{l}", [din, HC], f32)
            wsb[f"We{l}"] = load_sb(consts, f"We{l}", [cfg.EDGE_DIM, HC], b16)
            wsb[f"bl{l}"] = load_sb(consts, f"bl{l}", [HC, 1], f32)
            wsb[f"br{l}"] = load_sb(consts, f"br{l}", [HC, 1], f32)
            wsb[f"att{l}"] = load_sb(consts, f"att{l}", [P, HC], b16, bcast_p=P)
            wsb[f"g{l}"] = load_sb(consts, f"g{l}", [HC, 1], f32)
            wsb[f"be{l}"] = load_sb(consts, f"be{l}", [HC, 1], f32)
        wsb["W1"] = load_sb(consts, "W1", [HC, cfg.HS2], f32)
        wsb["W2"] = load_sb(consts, "W2", [cfg.HS2, 1], b16)
        wsb["gf"] = load_sb(consts, "gf", [cfg.HS2, 1], f32)
        wsb["bf"] = load_sb(consts, "bf", [cfg.HS2, 1], f32)
        wsb["b2"] = load_sb(consts, "b2", [1, 1], f32)

        # ---- resident edge metadata
        dstloc = load_sb(resident, "dstloc_w", [P, TT], b16)
        parity = load_sb(resident, "parity_w", [P, TT], mybir.dt.uint8)
        valid = load_sb(resident, "valid_w", [P, TT], b16)
        srcpair = load_sb(resident, "srcpair_w", [P, TT * 8], i16)
        ea_res = load_sb(resident, "eattr_em", [P, TT * cfg.EDGE_DIM], b16)
        # SBUF-resident xl pair table (reloaded per layer from the AllGather)
        if "dblbuf" in ablate:
            xl_sbs = [resident.tile([P, RANKS * P], b16, name=f"xl_sb{l}")
                      for l in range(2)]
        else:
            xl_sbs = [resident.tile([P, RANKS * P], b16, name="xl_sb")] * 2

        # ---- dram scratch
        xl_mine = [dram.tile([P, PAIRS], b16, name=f"xl_mine{l}") for l in range(2)]
        xl_pair = [dram.tile([cfg.CORES * PAIRS, P], b16, name=f"xl_pair{l}",
                             addr_space="Shared") for l in range(2)]
        st_in = dram.tile([HC, 2], f32, name="st_in")
        st_out = dram.tile([HC, 2], f32, name="st_out")

        # persistent per-layer activations
        act_pre = resident.tile([HC, NLPAD], f32, name="act_pre")
        act_fm = resident.tile([HC, NLPAD], f32, name="act_fm")
        xr_we = resident.tile([P, NBLK * HC], b16, name="xr_we")

        def bcast_f(ap, n, axis):
            """insert a stride-0 dim of size n at position axis of ap (free dims only)"""
            a = list(ap.ap)
            a.insert(axis, [0, n])
            return bass.AP(tensor=ap.tensor, offset=ap.offset, ap=a)

        # ================= node-side table builders =================
        def build_tables(l):
            """xl pair-table slice (HBM) + AllGather, xr_we blocks (SBUF)."""
            din = cfg.D_IN if l == 0 else HC
            with ExitStack() as c2:
                tp = c2.enter_context(tc.tile_pool(name=f"tb{l}", bufs=3))
                pp = c2.enter_context(tc.tile_pool(name=f"tbp{l}", bufs=2, space="PSUM"))
                if l == 0:
                    xin = tp.tile([cfg.D_IN, NLPAD], f32, name="xloc0", bufs=1)
                    nc.sync.dma_start(out=xin[:], in_=dp["xT_loc"][:])
                else:
                    xin = act_fm
                GN = 4  # local node tiles per group
                NT_LOC = NLPAD // P
                ngroups = NT_LOC // GN + (1 if NT_LOC % GN else 0)
                for g in range(ngroups):
                    nt0 = g * GN
                    gn = min(GN, NT_LOC - nt0)
                    cols = gn * P
                    ps_fm = pp.tile([HC, GN * P], f32, space="PSUM", name=f"psfm{l}")
                    nc.tensor.matmul(out=ps_fm[:, :cols], lhsT=wsb[f"Wl{l}"][:],
                                     rhs=xin[:, nt0 * P: nt0 * P + cols],
                                     start=True, stop=True)
                    fm_sb = tp.tile([HC, GN * P], b16, name=f"fmsb{l}")
                    nc.scalar.activation(fm_sb[:, :cols], ps_fm[:, :cols], AF.Identity,
                                         wsb[f"bl{l}"][:], 1.0)
                    ps_nm = pp.tile([P, GN, HC], b16, space="PSUM", name=f"psnm{l}")
                    for i in range(gn):
                        nc.tensor.transpose(out=ps_nm[:, i, :],
                                            in_=fm_sb[:, i * P:(i + 1) * P],
                                            identity=ident[:HC, :HC])
                    stage = tp.tile([P, GN, HC], b16, name=f"stage{l}")
                    nc.scalar.activation(stage[:, :gn, :], ps_nm[:, :gn, :], AF.Copy,
                                         0.0, 1.0)
                    # tile t holds nodes t*128..t*128+127; pair_local
                    # p_l = t*64 + p//2 -> tok = p_l & 127, k = p_l >> 7;
                    # DRAM (tok, k*128 + (p%2)*64 + f), tok-major rows of PAIRS
                    xp = xl_mine[l][:]
                    for i in range(gn):
                        t = nt0 + i
                        out_ap = bass.AP(
                            tensor=xp.tensor,
                            offset=xp.offset + (64 * (t % 2)) * PAIRS
                            + (t // 2) * P,
                            ap=[[PAIRS, 64], [HC, 2], [1, HC]])
                        nc.sync.dma_start(out=out_ap, in_=stage[:, i, :])
                if "nocoll" in ablate:
                    nc.gpsimd.dma_start(out=xl_pair[l][:PAIRS, :],
                                        in_=xl_mine[l][:])
                else:
                    nc.gpsimd.collective_compute(
                        "AllGather", OP.bypass,
                        replica_groups=[list(range(cfg.CORES))],
                        ins=[xl_mine[l][:]], outs=[xl_pair[l][:]])
                # xr_we blocks from local features (rows 0:NB); rows NB:128 = We
                for b in range(NBLK):
                    ps_b = pp.tile([HC, NB], f32, space="PSUM", name=f"psb{l}")
                    nc.tensor.matmul(out=ps_b[:], lhsT=wsb[f"Wr{l}"][:],
                                     rhs=xin[:, b * NB:(b + 1) * NB],
                                     start=True, stop=True)
                    xr_sb = tp.tile([HC, NB], b16, name=f"xrsb{l}")
                    nc.scalar.activation(xr_sb[:], ps_b[:], AF.Identity,
                                         wsb[f"br{l}"][:], 1.0)
                    ps_t = pp.tile([NB, HC], b16, space="PSUM", name=f"pst{l}")
                    nc.tensor.transpose(out=ps_t[:], in_=xr_sb[:],
                                        identity=ident[:HC, :HC])
                    nc.scalar.activation(xr_we[:NB, b * HC:(b + 1) * HC], ps_t[:],
                                         AF.Copy, 0.0, 1.0)
                nc.sync.dma_start(out=xr_we[NB:, :], in_=dp[f"We_rows{l}"][:])

        # ================= edge stage =================
        def edge_stage(l):
            with ExitStack() as c2:
                ep = c2.enter_context(tc.tile_pool(name=f"ed{l}", bufs=2))
                gp = c2.enter_context(tc.tile_pool(name=f"eg{l}", bufs=3))
                xp = c2.enter_context(tc.tile_pool(name=f"edx{l}", bufs=2, space="PSUM"))
                tp = c2.enter_context(tc.tile_pool(name=f"edo{l}", bufs=2, space="PSUM"))
                mp = c2.enter_context(tc.tile_pool(name=f"edm{l}", bufs=2, space="PSUM"))
                ag = c2.enter_context(tc.tile_pool(name=f"eda{l}", bufs=1, space="PSUM"))
                pt = c2.enter_context(tc.tile_pool(name=f"edq{l}", bufs=1, space="PSUM"))
                # load this layer's pair table into SBUF
                xl_sb = xl_sbs[l]
                if l == 1 and "dblbuf" not in ablate:
                    # WAR fence: act_fm readiness implies every layer-0 gather
                    # has fully drained its read of xl_sb
                    nc.gpsimd.dma_start(out=xl_sb[0:1, 0:1],
                                        in_=act_fm[0:1, 0:1])
                xl_ap = xl_pair[l][:]
                nc.gpsimd.dma_start(
                    out=xl_sb[:].rearrange("p (c x) -> p c x", c=cfg.CORES),
                    in_=bass.AP(tensor=xl_ap.tensor, offset=xl_ap.offset,
                                ap=[[PAIRS, P], [P * PAIRS, cfg.CORES],
                                    [1, PAIRS]]))
                ngr = math.ceil(T_B / 8)
                base, rem = divmod(T_B, ngr)
                groups, t0 = [], 0
                for i in range(ngr):
                    gt = base + (1 if i < rem else 0)
                    groups.append((t0, gt))
                    t0 += gt
                for b in range(NBLK):
                    xlT = ep.tile([P, 1, ET], b16, name=f"xlT{l}")
                    if "nogather" in ablate:
                        nc.sync.dma_start(
                            out=xlT[:],
                            in_=xl_pair[l][:].rearrange(
                                "(a b) p -> a (b p)", a=P)[:, :ET])
                    elif "hbmgather" in ablate:
                        nc.gpsimd.dma_gather(
                            out_ap=xlT[:], in_ap=xl_pair[l][:],
                            idxs_ap=srcpair[:, b * (ET // 16):(b + 1) * (ET // 16)],
                            num_idxs=ET, num_idxs_reg=ET, elem_size=P,
                            transpose=True, single_packet=False)
                    else:
                        nc.gpsimd.dma_gather(
                            out_ap=xlT[:], in_ap=xl_sb[:],
                            idxs_ap=srcpair[:, b * (ET // 16):(b + 1) * (ET // 16)],
                            num_idxs=ET, num_idxs_reg=ET, elem_size=P,
                            transpose=True,
                            single_packet=("sp" in ablate),
                            queue_num=b % nq,
                            sbuf_tokens_per_rank=P,
                            sbuf_free_dim_per_rank=256)
                    if "gatheronly" in ablate:
                        continue
                    agg = ag.tile([P, PAY], f32, space="PSUM", name=f"agg{l}")
                    for gi, (t0, gt) in enumerate(groups):
                        cols = slice(b * T_B + t0, b * T_B + t0 + gt)
                        # edge-major one-hot over dst slots (for aggregation)
                        oh_g = gp.tile([P, 8, P], b16, name=f"oh{l}")
                        nc.vector.tensor_tensor(
                            out=oh_g[:, :gt, :],
                            in0=bcast_f(dstloc[:, cols], P, 2),
                            in1=bcast_f(iotaB[:], gt, 1),
                            op=OP.is_equal)
                        nc.vector.tensor_copy(
                            oh_g[:, :gt, NB:],
                            ea_res[:].rearrange("p (t f) -> p t f",
                                                f=cfg.EDGE_DIM)[:, cols, :])
                        # slot-major one-hot (for xr select): PE transpose
                        ps_o = tp.tile([P, 8, P], b16, space="PSUM", name=f"pso{l}")
                        for k in range(gt):
                            nc.tensor.transpose(out=ps_o[:, k, :],
                                                in_=oh_g[:, k, :],
                                                identity=ident[:])
                        ohT_g = gp.tile([P, 8, P], b16, name=f"ohT{l}")
                        nc.vector.tensor_copy(ohT_g[:, :gt, :], ps_o[:, :gt, :])
                        # edge-major gathered pairs + parity select
                        ps_x = xp.tile([P, 8, P], b16, space="PSUM", name=f"psx{l}")
                        for k in range(gt):
                            nc.tensor.transpose(
                                out=ps_x[:, k, :],
                                in_=xlT[:, 0, (t0 + k) * P:(t0 + k + 1) * P],
                                identity=ident[:])
                        xlsel_t = gp.tile([P, 8, HC + 2], b16, name=f"xls{l}")
                        xlsel_g = xlsel_t[:, :, :HC]
                        nc.scalar.activation(xlsel_t[:, :gt, :HC],
                                             ps_x[:, :gt, :HC], AF.Copy, 0.0, 1.0)
                        xlodd_t = gp.tile([P, 8, HC + 2], b16, name=f"xlo{l}")
                        nc.scalar.activation(xlodd_t[:, :gt, :HC],
                                             ps_x[:, :gt, HC:], AF.Copy, 0.0, 1.0)
                        nc.vector.copy_predicated(
                            xlsel_t[:, :gt, :HC],
                            bcast_f(parity[:, cols], HC, 2),
                            xlodd_t[:, :gt, :HC])
                        if "noedge" in ablate:
                            continue
                        # messages: m = (xr[dst] + eattr@We) + xl[src] in PSUM
                        psm = mp.tile([P, 8, HC], f32, space="PSUM", name=f"psm{l}")
                        for k in range(gt):
                            nc.tensor.matmul(
                                out=psm[:, k, :], lhsT=ohT_g[:, k, :],
                                rhs=xr_we[:, b * HC:(b + 1) * HC],
                                start=True, stop=False)
                            nc.tensor.matmul(
                                out=psm[:, k, :], lhsT=ident[:],
                                rhs=xlsel_t[:, k, :HC],
                                start=False, stop=True)
                        # lrelu(x) = x + 0.8*relu(-x), then * att
                        r2n = gp.tile([P, 8, HC], b16, name=f"r2n{l}")
                        nc.scalar.activation(r2n[:, :gt, :], psm[:, :gt, :],
                                             AF.Relu, 0.0, -1.0)
                        mlr = gp.tile([P, 8, HC], b16, name=f"mlr{l}")
                        nc.vector.scalar_tensor_tensor(
                            out=mlr[:, :gt, :], in0=r2n[:, :gt, :],
                            scalar=0.8, in1=psm[:, :gt, :],
                            op0=OP.mult, op1=OP.add)
                        nc.vector.tensor_tensor(
                            out=mlr[:, :gt, :], in0=mlr[:, :gt, :],
                            in1=bcast_f(wsb[f"att{l}"][:], gt, 1), op=OP.mult)
                        logit = gp.tile([P, 8, H], f32, name=f"lg{l}")
                        nc.vector.tensor_reduce(
                            out=logit[:, :gt, :],
                            in_=mlr[:, :gt, :].rearrange("p t (h c) -> p t h c",
                                                         h=H),
                            axis=mybir.AxisListType.X, op=OP.add)
                        pay = gp.tile([P, 8, PAY], b16, name=f"pay{l}")
                        nc.scalar.activation(pay[:, :gt, HC:], logit[:, :gt, :],
                                             AF.Exp, 0.0, 1.0)
                        nc.vector.tensor_tensor(
                            out=pay[:, :gt, HC:], in0=pay[:, :gt, HC:],
                            in1=bcast_f(valid[:, cols], H, 2),
                            op=OP.mult)
                        nc.vector.tensor_tensor(
                            out=pay[:, :gt, :HC].rearrange(
                                "p t (h c) -> p t h c", h=H),
                            in0=xlsel_t[:, :gt, :HC].rearrange(
                                "p t (h c) -> p t h c", h=H),
                            in1=bcast_f(pay[:, :gt, HC:], C, 3), op=OP.mult)
                        for k in range(gt):
                            nc.tensor.matmul(
                                out=agg[:], lhsT=oh_g[:, k, :], rhs=pay[:, k, :],
                                start=(gi == 0 and k == 0),
                                stop=(gi == ngr - 1 and k == gt - 1))
                    if "noedge" in ablate:
                        continue
                    dena = ep.tile([NB, H], f32, name=f"dena{l}")
                    nc.scalar.activation(dena[:], agg[:NB, HC:], AF.Identity,
                                         c_tiny[:NB, :], 1.0)
                    denr = ep.tile([NB, H], f32, name=f"denr{l}")
                    nc.vector.reciprocal(denr[:], dena[:])
                    xn = ep.tile([NB, HC], f32, name=f"xn{l}")
                    nc.vector.tensor_tensor(
                        out=xn[:].rearrange("n (h c) -> n h c", h=H),
                        in0=agg[:NB, :HC].rearrange("n (h c) -> n h c", h=H),
                        in1=bcast_f(denr[:], C, 2), op=OP.mult)
                    ps_t = pt.tile([HC, NB], f32, space="PSUM", name=f"epst{l}")
                    nc.tensor.transpose(out=ps_t[:], in_=xn[:],
                                        identity=identF[:NB, :NB])
                    nc.scalar.activation(act_pre[:, b * NB:(b + 1) * NB], ps_t[:],
                                         AF.Copy, 0.0, 1.0)

        # ================= BN + ELU (feature-major, per-partition channels) ====
        def bn_elu(x_sb, F, ncols, nlp, g_ap, be_ap, tag, ach=2048, bbufs=2):
            """x_sb [F, nlp] f32 tile; stats over first ncols cols; applies
            ELU(BN(x)) in place. Uses AllReduce for global stats."""
            with ExitStack() as c2:
                bp = c2.enter_context(tc.tile_pool(name=f"bn{tag}", bufs=bbufs))
                ssum = bp.tile([F, 2], f32, name=f"ssum{tag}")
                nc.vector.tensor_reduce(out=ssum[:, 0:1], in_=x_sb[:, :ncols],
                                        axis=mybir.AxisListType.X, op=OP.add)
                BCH = 1024
                nbch = math.ceil(ncols / BCH)
                sq_parts = bp.tile([F, nbch], f32, name=f"sqp{tag}")
                for bi in range(nbch):
                    c0 = bi * BCH
                    cw = min(BCH, ncols - c0)
                    sq = bp.tile([F, BCH], f32, name=f"sq{tag}")
                    nc.scalar.activation(sq[:, :cw], x_sb[:, c0:c0 + cw],
                                         AF.Square, 0.0, 1.0,
                                         accum_out=sq_parts[:, bi:bi + 1])
                nc.vector.tensor_reduce(out=ssum[:, 1:2], in_=sq_parts[:],
                                        axis=mybir.AxisListType.X, op=OP.add)
                nc.sync.dma_start(out=st_in[:F, :], in_=ssum[:])
                if "nocoll" in ablate:
                    nc.gpsimd.dma_start(out=st_out[:F, :], in_=st_in[:F, :])
                else:
                    nc.gpsimd.collective_compute(
                        "AllReduce", OP.add,
                        replica_groups=[list(range(cfg.CORES))],
                        ins=[st_in[:F, :]], outs=[st_out[:F, :]])
                gs = bp.tile([F, 2], f32, name=f"gs{tag}")
                nc.sync.dma_start(out=gs[:], in_=st_out[:F, :])
                mean = bp.tile([F, 1], f32, name=f"mean{tag}")
                nc.scalar.activation(mean[:], gs[:, 0:1], AF.Copy, 0.0, 1.0 / cfg.N)
                msq = bp.tile([F, 1], f32, name=f"msq{tag}")
                nc.scalar.activation(msq[:], gs[:, 1:2], AF.Copy, 0.0, 1.0 / cfg.N)
                # var = msq - mean^2 (biased)
                m2 = bp.tile([F, 1], f32, name=f"m2{tag}")
                nc.vector.tensor_tensor(out=m2[:], in0=mean[:], in1=mean[:],
                                        op=OP.mult)
                var = bp.tile([F, 1], f32, name=f"var{tag}")
                nc.vector.tensor_tensor(out=var[:], in0=msq[:], in1=m2[:],
                                        op=OP.subtract)
                vare = bp.tile([F, 1], f32, name=f"vare{tag}")
                nc.scalar.activation(vare[:], var[:], AF.Identity, c_eps[:F, :], 1.0)
                vrec = bp.tile([F, 1], f32, name=f"vrec{tag}")
                nc.vector.reciprocal(vrec[:], vare[:])
                rstd = bp.tile([F, 1], f32, name=f"rstd{tag}")
                nc.scalar.activation(rstd[:], vrec[:], AF.Sqrt, 0.0, 1.0)
                scl = bp.tile([F, 1], f32, name=f"scl{tag}")
                nc.vector.tensor_tensor(out=scl[:], in0=g_ap, in1=rstd[:], op=OP.mult)
                sht = bp.tile([F, 1], f32, name=f"sht{tag}")
                nc.vector.tensor_tensor(out=sht[:], in0=mean[:], in1=scl[:], op=OP.mult)
                nc.vector.tensor_tensor(out=sht[:], in0=be_ap, in1=sht[:], op=OP.subtract)
                ACH = ach
                nach = math.ceil(nlp / ACH)
                for ai in range(nach):
                    c0 = ai * ACH
                    cw = min(ACH, nlp - c0)
                    y = bp.tile([F, ACH], f32, name=f"y{tag}")
                    nc.vector.scalar_tensor_tensor(
                        out=y[:, :cw], in0=x_sb[:, c0:c0 + cw], scalar=scl[:],
                        in1=bass.AP(tensor=sht.tensor, offset=sht[:].offset,
                                    ap=[sht[:].ap[0], [0, cw]]),
                        op0=OP.mult, op1=OP.add)
                    r = bp.tile([F, ACH], f32, name=f"r{tag}")
                    nc.scalar.activation(r[:, :cw], y[:, :cw], AF.Relu, 0.0, 1.0)
                    ng = bp.tile([F, ACH], f32, name=f"ng{tag}")
                    nc.vector.tensor_tensor(out=ng[:, :cw], in0=y[:, :cw],
                                            in1=r[:, :cw], op=OP.subtract)
                    eg = bp.tile([F, ACH], f32, name=f"eg{tag}")
                    nc.scalar.activation(eg[:, :cw], ng[:, :cw], AF.Exp, 0.0, 1.0)
                    nc.vector.scalar_tensor_tensor(
                        out=x_sb[:, c0:c0 + cw], in0=eg[:, :cw], scalar=-1.0,
                        in1=r[:, :cw], op0=OP.add, op1=OP.add)

        # ================= main program =================
        nc.vector.memset(act_pre[:], 0.01)
        if ablate & {"noedge", "noedgestage", "minimal", "empty"}:
            nc.vector.tensor_copy(act_fm[:], act_pre[:])
        for l in range(2):
            if "empty" in ablate:
                break
            if "minimal" not in ablate:
                build_tables(l)
                if "noedgestage" not in ablate:
                    edge_stage(l)
            if dbg_p is not None and l == int("debug2" in ablate) \
                    and "debug3" not in ablate:
                nc.sync.dma_start(out=dbg_p[:], in_=act_pre[:])
            bn_elu(act_pre, HC, NL, NLP, wsb[f"g{l}"][:], wsb[f"be{l}"][:], f"l{l}",
                   ach=1024)
            nc.vector.tensor_copy(act_fm[:], act_pre[:])
            if dbg_p is not None and l == 1 and "debug3" in ablate:
                nc.sync.dma_start(out=dbg_p[:], in_=act_fm[:])

        # ---- head: x @ W1 -> BN -> ELU -> @ W2 -> 5*tanh
        with ExitStack() as c2:
            hp = c2.enter_context(tc.tile_pool(name="head", bufs=1))
            hpp = c2.enter_context(tc.tile_pool(name="headp", bufs=2, space="PSUM"))
            x3 = hp.tile([cfg.HS2, NLP], b16, name="x3")
            CH = 512
            nch = math.ceil(NLP / CH)
            for ci in range(nch):
                c0 = ci * CH
                cw = min(CH, NLP - c0)
                psh = hpp.tile([cfg.HS2, CH], f32, space="PSUM", name="psh")
                nc.tensor.matmul(out=psh[:, :cw], lhsT=wsb["W1"][:],
                                 rhs=act_fm[:, c0:c0 + cw], start=True, stop=True)
                nc.scalar.activation(x3[:, c0:c0 + cw], psh[:, :cw], AF.Copy,
                                     0.0, 1.0)
            bn_elu(x3, cfg.HS2, NL, NLP, wsb["gf"][:], wsb["bf"][:], "hd",
                   ach=1024, bbufs=1)
            for ci in range(nch):
                c0 = ci * CH
                if c0 >= NL:
                    break
                cw = min(CH, NLP - c0)
                cv = min(cw, NL - c0)
                pso = hpp.tile([1, CH], f32, space="PSUM", name="pso")
                nc.tensor.matmul(out=pso[:, :cw], lhsT=wsb["W2"][:],
                                 rhs=x3[:, c0:c0 + cw], start=True, stop=True)
                th = hp.tile([1, CH], f32, name="th", bufs=2)
                nc.scalar.activation(th[:, :cw], pso[:, :cw], AF.Tanh,
                                     wsb["b2"][:], 1.0)
                nc.vector.tensor_scalar(out=th[:, :cw], in0=th[:, :cw],
                                        scalar1=5.0, scalar2=None, op0=OP.mult)
                nc.sync.dma_start(out=out_p[:, c0:c0 + cv], in_=th[:, :cv])

    nc.compile()
    return nc


# ---------------------------------------------------------------- entry

_CACHE = {}


def _get_built(cfg: Cfg, T_B: int):
    key = (cfg.N, cfg.E, T_B)
    if key not in _CACHE:
        _CACHE[key] = build(cfg, T_B)
    return _CACHE[key]


def run(cfg: Cfg, inputs: dict):
    w = prep_weights(cfg, inputs)
    in_maps, T_B = prep(cfg, inputs["data_x"], inputs["data_edge_index"],
                        inputs["data_edge_attr"], w)
    nc = _get_built(cfg, T_B)
    res = run_bass_kernel_spmd(nc, in_maps, core_ids=list(range(cfg.CORES)))
    out = np.concatenate([np.asarray(res.results[c]["out"]).reshape(-1)
                          for c in range(cfg.CORES)])
    return out.reshape(cfg.N, 1).astype(np.float32)


def kernel(**inputs):
    return run(FULL, inputs)


# ---------------------------------------------------------------- timing

def time_kernel(inputs, iters=20):
    """Build the jitted 8-core executable once, run it `iters` times with
    device-resident inputs, return average per-execution wall time in ns."""
    import time
    import jax
    from jax.experimental.shard_map import shard_map
    from jax.sharding import Mesh, PartitionSpec, NamedSharding
    from concourse import bass2jax, mybir as _mb

    cfg = FULL
    w = prep_weights(cfg, inputs)
    in_maps, T_B = prep(cfg, inputs["data_x"], inputs["data_edge_index"],
                        inputs["data_edge_attr"], w)
    nc = _get_built(cfg, T_B)
    bass2jax.install_neuronx_cc_hook()
    n_cores = cfg.CORES
    partition_name = nc.partition_id_tensor.name if nc.partition_id_tensor else None
    in_names, out_names, out_avals, zero_outs = [], [], [], []
    for alloc in nc.m.functions[0].allocations:
        if not isinstance(alloc, _mb.MemoryLocationSet):
            continue
        name = alloc.memorylocations[0].name
        if alloc.kind == "ExternalInput":
            if name != partition_name:
                in_names.append(name)
        elif alloc.kind == "ExternalOutput":
            out_names.append(name)
            shape = tuple(alloc.tensor_shape)
            dtype = _mb.dt.np(alloc.dtype)
            out_avals.append(jax.core.ShapedArray(shape, dtype))
            zero_outs.append(np.zeros(shape, dtype))
    n_params = len(in_names)
    all_in = list(in_names) + list(out_names)
    if partition_name is not None:
        all_in.append(partition_name)

    def _body(*args):
        operands = list(args)
        if partition_name is not None:
            operands.append(bass2jax.partition_id_tensor())
        outs = bass2jax._bass_exec_p.bind(
            *operands,
            out_avals=tuple(out_avals),
            in_names=tuple(all_in),
            out_names=tuple(out_names),
            lowering_input_output_aliases=(),
            sim_require_finite=True,
            sim_require_nnan=True,
            nc=nc,
        )
        return tuple(outs)

    devices = jax.devices()[:n_cores]
    mesh = Mesh(np.asarray(devices), ("core",))
    n_outs = len(out_names)
    in_specs = (PartitionSpec("core"),) * (n_params + n_outs)
    out_specs = (PartitionSpec("core"),) * n_outs
    sharded = jax.jit(
        shard_map(_body, mesh=mesh, in_specs=in_specs, out_specs=out_specs,
                  check_rep=False),
        keep_unused=True)
    sh = NamedSharding(mesh, PartitionSpec("core"))
    concat_in = [
        jax.device_put(
            np.concatenate([np.asarray(in_maps[c][nm]) for c in range(n_cores)],
                           axis=0), sh)
        for nm in in_names]
    concat_zeros = [
        jax.device_put(np.zeros((n_cores * z.shape[0], *z.shape[1:]), z.dtype), sh)
        for z in zero_outs]
    outs = sharded(*concat_in, *concat_zeros)  # warm-up (compiles)
    jax.block_until_ready(outs)
    for _ in range(15):  # steady-state warm-up
        outs = sharded(*concat_in, *concat_zeros)
    jax.block_until_ready(outs)
    iters = max(iters, 300)
    t0 = time.perf_counter()
    for _ in range(iters):
        outs = sharded(*concat_in, *concat_zeros)
    jax.block_until_ready(outs)
    t1 = time.perf_counter()
    return (t1 - t0) / iters * 1e9


# revision 48
# speedup vs baseline: 1.0267x; 1.0267x over previous
"""Trainium2 Bass kernel for CustomGNNvA (2-layer GATv2 + BN/ELU + MLP head).

Self-contained: hardcodes all shapes. Edge-parallel sharding by destination
node range across 8 NeuronCores; per-core local softmax-aggregation via
one-hot matmuls; cross-core collectives for BN stats (AllReduce) and the
per-layer xl gather table (AllGather of per-core slices).

v2 vs baseline:
 - node transform split across cores (1/8 each) + AllGather of the xl pair
   table (replaces per-core full-N transform + activation AllGather)
 - one-hot dst-select matrix built on-chip (PE transpose of the DVE-built
   one-hot) instead of streaming a 116-row one-hot table from HBM
 - LeakyReLU as a single DVE scalar_tensor_tensor max(0.2x, x)
 - payload multiply on DVE (was GPSIMD), parity select as one DVE select
 - NB=128 dst nodes per block, exp held in bf16 inside the payload tile
"""
import sys

sys.path.insert(0, "/opt/trn_rl_repo")

import math
from contextlib import ExitStack
from dataclasses import dataclass

import numpy as np
import ml_dtypes

from concourse import bass, mybir, tile, bacc
from concourse.bass_utils import run_bass_kernel_spmd
from concourse.masks import make_identity

BF16 = ml_dtypes.bfloat16
P = 128


@dataclass
class Cfg:
    N: int = 50000
    E: int = 1600000
    D_IN: int = 128
    H: int = 4
    C: int = 16
    HC: int = 64
    HS2: int = 64
    EDGE_DIM: int = 4
    EPS: float = 1e-5
    CORES: int = 8
    NB: int = 124          # dst nodes per block (one-hot rows; +4 eattr rows)

    @property
    def NL(self):  # nodes per core
        return self.N // self.CORES

    @property
    def NBLK(self):  # dst blocks per core
        return math.ceil(self.NL / self.NB)

    @property
    def NLOC(self):  # padded local node count
        return self.NBLK * self.NB

    @property
    def NLPAD(self):  # local nodes padded to 128-node table tiles
        return math.ceil(self.NLOC / 128) * 128

    @property
    def PAIRS(self):  # pair rows per core (padded to table tiles)
        return self.NLPAD // 2


FULL = Cfg()


# ---------------------------------------------------------------- CPU prep

def prep(cfg: Cfg, data_x, data_edge_index, data_edge_attr, weights: dict):
    """Shard + reorder edges; build all per-core input arrays (layout only)."""
    src = np.asarray(data_edge_index[0]).astype(np.int64)
    dst = np.asarray(data_edge_index[1]).astype(np.int64)
    eattr = np.asarray(data_edge_attr, np.float32)
    NL, NB, NBLK, CORES = cfg.NL, cfg.NB, cfg.NBLK, cfg.CORES

    core = dst // NL
    dstloc = dst % NL
    blk = dstloc // NB
    dstin = dstloc % NB
    gkey = core * NBLK + blk
    counts = np.bincount(gkey, minlength=CORES * NBLK)
    T_B = max(1, int(math.ceil(counts.max() / P)))
    ET = T_B * P                       # padded edges per block
    TT = NBLK * T_B                    # tiles per core
    TE = NBLK * ET                     # padded edges per core

    # position of each edge in its core's padded array
    order = np.argsort(gkey, kind="stable")
    within = np.arange(cfg.E) - np.concatenate([[0], np.cumsum(counts)])[gkey[order]]
    pos = np.empty(cfg.E, np.int64)
    pos[order] = (blk[order] * ET) + within

    # global pair row of a source node (pair table is per-core slices of
    # PAIRS rows concatenated)
    pairrow = (src // NL) * cfg.PAIRS + (src % NL) // 2

    in_maps = [dict() for _ in range(CORES)]
    for c in range(CORES):
        sel = core == c
        p_c = pos[sel]
        row_e = np.zeros(TE, np.int64)
        par_e = np.zeros(TE, np.int64)
        dstin_e = np.zeros(TE, np.int64)
        valid_e = np.zeros(TE, np.float32)
        eattr_e = np.zeros((TE, cfg.EDGE_DIM), np.float32)
        row_e[p_c] = pairrow[sel]
        par_e[p_c] = src[sel] & 1
        dstin_e[p_c] = dstin[sel]
        valid_e[p_c] = 1.0
        eattr_e[p_c] = eattr[sel]

        def wrap(x):  # [TE] -> [128, TT]; edge j=(t*128+p) at [p, t]
            return np.ascontiguousarray(x.reshape(TT, P).T)

        m = in_maps[c]
        m["dstloc_w"] = wrap(dstin_e.astype(np.float32)).astype(BF16)
        m["parity_w"] = wrap(par_e.astype(np.float32)).astype(np.uint8)
        m["valid_w"] = wrap(valid_e).astype(BF16)
        # gather indices: per chunk, wrapped-16 and replicated to 128 partitions
        idx = row_e.astype(np.int16).reshape(NBLK, ET)
        iw = np.zeros((NBLK, P, ET // 16), np.int16)
        for g in range(8):
            iw[:, g * 16:(g + 1) * 16, :] = idx.reshape(NBLK, ET // 16, 16).transpose(0, 2, 1)
        m["srcpair_w"] = np.ascontiguousarray(iw.transpose(1, 0, 2).reshape(P, NBLK * (ET // 16)))
        # edge-major eattr, wrapped like dstloc: [128, TT, 4]
        m["eattr_em"] = np.ascontiguousarray(
            eattr_e.reshape(TT, P, cfg.EDGE_DIM).transpose(1, 0, 2)
            .reshape(P, TT * cfg.EDGE_DIM)).astype(BF16)

        xl = np.zeros((cfg.D_IN, cfg.NLPAD), np.float32)
        xl[:, :NL] = np.asarray(data_x, np.float32).T[:, c * NL:(c + 1) * NL]
        m["xT_loc"] = xl
        for k, v in weights.items():
            m[k] = v
    return in_maps, T_B


def prep_weights(cfg: Cfg, inp: dict):
    w = {}
    for l in range(2):
        w[f"Wl{l}"] = np.asarray(inp[f"Wl{l}"], np.float32)
        w[f"Wr{l}"] = np.asarray(inp[f"Wr{l}"], np.float32)
        w[f"We{l}"] = np.asarray(inp[f"We{l}"], np.float32).astype(BF16)
        w[f"We_rows{l}"] = np.tile(w[f"We{l}"], (1, cfg.NBLK))
        w[f"bl{l}"] = np.asarray(inp[f"bl{l}"], np.float32).reshape(-1, 1)
        w[f"br{l}"] = np.asarray(inp[f"br{l}"], np.float32).reshape(-1, 1)
        w[f"att{l}"] = np.asarray(inp[f"att{l}"], np.float32).reshape(1, -1).astype(BF16)
        w[f"g{l}"] = np.asarray(inp[f"g{l}"], np.float32).reshape(-1, 1)
        w[f"be{l}"] = np.asarray(inp[f"be{l}"], np.float32).reshape(-1, 1)
    w["W1"] = np.asarray(inp["W1"], np.float32)
    w["W2"] = np.asarray(inp["W2"], np.float32).astype(BF16)
    w["gf"] = np.asarray(inp["gf"], np.float32).reshape(-1, 1)
    w["bf"] = np.asarray(inp["bf"], np.float32).reshape(-1, 1)
    w["b2"] = np.asarray(inp["b2"], np.float32).reshape(1, 1)
    return w


# ---------------------------------------------------------------- builder

def build(cfg: Cfg, T_B: int, ablate: frozenset = frozenset()):
    nq = 4 if "q4" in ablate else (2 if "q2" in ablate else 1)
    nc = bacc.Bacc(None, target_bir_lowering=False, debug=False,
                   num_devices=cfg.CORES, num_swdge_queues=nq)
    f32, b16, i16 = mybir.dt.float32, mybir.dt.bfloat16, mybir.dt.int16
    AF = mybir.ActivationFunctionType
    OP = mybir.AluOpType
    NB, NBLK, NL, HC, H, C = cfg.NB, cfg.NBLK, cfg.NL, cfg.HC, cfg.H, cfg.C
    ET = T_B * P
    TT = NBLK * T_B
    NLP = cfg.NLPAD
    PAIRS = cfg.PAIRS
    PAY = HC + H  # 68

    # ---- dram parameters
    dp = {}
    def param(name, shape, dt):
        dp[name] = nc.declare_dram_parameter(name, list(shape), dt, isOutput=False)
        return dp[name]

    RANKS = cfg.CORES * PAIRS // P
    NLPAD = cfg.NLPAD
    if "fewparams" in ablate:
        param("xT_loc", [cfg.D_IN, NLPAD], f32)
        out_p = nc.declare_dram_parameter("out", [1, NL], f32, isOutput=True)
        with tile.TileContext(nc) as tc, ExitStack() as ctx:
            pool = ctx.enter_context(tc.tile_pool(name="fp", bufs=1))
            t = pool.tile([1, NL], f32, name="t")
            nc.sync.dma_start(out=t[:], in_=dp["xT_loc"][0:1, :NL])
            nc.sync.dma_start(out=out_p[:], in_=t[:])
        nc.compile()
        return nc
    param("dstloc_w", [P, TT], b16)
    param("parity_w", [P, TT], mybir.dt.uint8)
    param("valid_w", [P, TT], b16)
    param("srcpair_w", [P, TT * 8], i16)
    param("eattr_em", [P, TT * cfg.EDGE_DIM], b16)
    param("xT_loc", [cfg.D_IN, NLPAD], f32)
    for l in range(2):
        din = cfg.D_IN if l == 0 else HC
        param(f"Wl{l}", [din, HC], f32)
        param(f"Wr{l}", [din, HC], f32)
        param(f"We{l}", [cfg.EDGE_DIM, HC], b16)
        param(f"We_rows{l}", [cfg.EDGE_DIM, NBLK * HC], b16)
        param(f"bl{l}", [HC, 1], f32)
        param(f"br{l}", [HC, 1], f32)
        param(f"att{l}", [1, HC], b16)
        param(f"g{l}", [HC, 1], f32)
        param(f"be{l}", [HC, 1], f32)
    param("W1", [HC, cfg.HS2], f32)
    param("W2", [cfg.HS2, 1], b16)
    param("gf", [cfg.HS2, 1], f32)
    param("bf", [cfg.HS2, 1], f32)
    param("b2", [1, 1], f32)
    out_p = nc.declare_dram_parameter("out", [1, NL], f32, isOutput=True)
    dbg_p = None
    if "debug" in ablate:
        dbg_p = nc.declare_dram_parameter("dbg", [cfg.HC, cfg.NLPAD], f32,
                                          isOutput=True)

    with tile.TileContext(nc) as tc, ExitStack() as ctx:
        consts = ctx.enter_context(tc.tile_pool(name="consts", bufs=1))
        resident = ctx.enter_context(tc.tile_pool(name="resident", bufs=1))
        dram = ctx.enter_context(tc.tile_pool(name="dram", bufs=1, space="DRAM"))

        # ---- constants in SBUF
        ident = consts.tile([P, P], b16, name="ident")
        make_identity(nc, ident[:])
        identF = consts.tile([P, P], f32, name="identF")
        make_identity(nc, identF[:])
        iota32 = consts.tile([P, P], mybir.dt.int32, name="iota32")
        nc.gpsimd.iota(iota32[:], pattern=[[1, P]], base=0, channel_multiplier=0)
        iotaB = consts.tile([P, P], b16, name="iotaB")
        nc.vector.tensor_copy(iotaB[:], iota32[:])
        c_tiny = consts.tile([P, 1], f32, name="c_tiny")
        nc.vector.memset(c_tiny[:], 1e-16)
        c_eps = consts.tile([P, 1], f32, name="c_eps")
        nc.vector.memset(c_eps[:], cfg.EPS)

        def load_sb(pool, name, shape, dt, bcast_p=None):
            t = pool.tile(list(shape), dt, name=f"sb_{name}")
            srcap = dp[name][:]
            if bcast_p is not None:
                srcap = bass.AP(tensor=srcap.tensor, offset=srcap.offset,
                                ap=[[0, bcast_p]] + srcap.ap[1:])
            nc.sync.dma_start(out=t[:], in_=srcap)
            return t

        wsb = {}
        for l in range(2):
            din = cfg.D_IN if l == 0 else HC
            wsb[f"Wl{l}"] = load_sb(consts, f"Wl{l}", [din, HC], f32)
            wsb[f"Wr{l}"] = load_sb(consts, f"Wr{l}", [din, HC], f32)
            wsb[f"We{l}"] = load_sb(consts, f"We{l}", [cfg.EDGE_DIM, HC], b16)
            wsb[f"bl{l}"] = load_sb(consts, f"bl{l}", [HC, 1], f32)
            wsb[f"br{l}"] = load_sb(consts, f"br{l}", [HC, 1], f32)
            wsb[f"att{l}"] = load_sb(consts, f"att{l}", [P, HC], b16, bcast_p=P)
            wsb[f"g{l}"] = load_sb(consts, f"g{l}", [HC, 1], f32)
            wsb[f"be{l}"] = load_sb(consts, f"be{l}", [HC, 1], f32)
        wsb["W1"] = load_sb(consts, "W1", [HC, cfg.HS2], f32)
        wsb["W2"] = load_sb(consts, "W2", [cfg.HS2, 1], b16)
        wsb["gf"] = load_sb(consts, "gf", [cfg.HS2, 1], f32)
        wsb["bf"] = load_sb(consts, "bf", [cfg.HS2, 1], f32)
        wsb["b2"] = load_sb(consts, "b2", [1, 1], f32)

        # ---- resident edge metadata
        dstloc = load_sb(resident, "dstloc_w", [P, TT], b16)
        parity = load_sb(resident, "parity_w", [P, TT], mybir.dt.uint8)
        valid = load_sb(resident, "valid_w", [P, TT], b16)
        srcpair = load_sb(resident, "srcpair_w", [P, TT * 8], i16)
        ea_res = load_sb(resident, "eattr_em", [P, TT * cfg.EDGE_DIM], b16)
        # SBUF-resident xl pair table (reloaded per layer from the AllGather)
        if "dblbuf" in ablate:
            xl_sbs = [resident.tile([P, RANKS * P], b16, name=f"xl_sb{l}")
                      for l in range(2)]
        else:
            xl_sbs = [resident.tile([P, RANKS * P], b16, name="xl_sb")] * 2

        # ---- dram scratch
        xl_mine = [dram.tile([P, PAIRS], b16, name=f"xl_mine{l}") for l in range(2)]
        xl_pair = [dram.tile([cfg.CORES * PAIRS, P], b16, name=f"xl_pair{l}",
                             addr_space="Shared") for l in range(2)]
        st_in = dram.tile([HC, 2], f32, name="st_in")
        st_out = dram.tile([HC, 2], f32, name="st_out")

        # persistent per-layer activations
        act_pre = resident.tile([HC, NLPAD], f32, name="act_pre")
        act_fm = resident.tile([HC, NLPAD], f32, name="act_fm")
        xr_we = resident.tile([P, NBLK * HC], b16, name="xr_we")

        def bcast_f(ap, n, axis):
            """insert a stride-0 dim of size n at position axis of ap (free dims only)"""
            a = list(ap.ap)
            a.insert(axis, [0, n])
            return bass.AP(tensor=ap.tensor, offset=ap.offset, ap=a)

        # ================= node-side table builders =================
        def build_tables(l):
            """xl pair-table slice (HBM) + AllGather, xr_we blocks (SBUF)."""
            din = cfg.D_IN if l == 0 else HC
            with ExitStack() as c2:
                tp = c2.enter_context(tc.tile_pool(name=f"tb{l}", bufs=3))
                pp = c2.enter_context(tc.tile_pool(name=f"tbp{l}", bufs=2, space="PSUM"))
                if l == 0:
                    xin = tp.tile([cfg.D_IN, NLPAD], f32, name="xloc0", bufs=1)
                    nc.sync.dma_start(out=xin[:], in_=dp["xT_loc"][:])
                else:
                    xin = act_fm
                GN = 4  # local node tiles per group
                NT_LOC = NLPAD // P
                ngroups = NT_LOC // GN + (1 if NT_LOC % GN else 0)
                for g in range(ngroups):
                    nt0 = g * GN
                    gn = min(GN, NT_LOC - nt0)
                    cols = gn * P
                    ps_fm = pp.tile([HC, GN * P], f32, space="PSUM", name=f"psfm{l}")
                    nc.tensor.matmul(out=ps_fm[:, :cols], lhsT=wsb[f"Wl{l}"][:],
                                     rhs=xin[:, nt0 * P: nt0 * P + cols],
                                     start=True, stop=True)
                    fm_sb = tp.tile([HC, GN * P], b16, name=f"fmsb{l}")
                    nc.scalar.activation(fm_sb[:, :cols], ps_fm[:, :cols], AF.Identity,
                                         wsb[f"bl{l}"][:], 1.0)
                    ps_nm = pp.tile([P, GN, HC], b16, space="PSUM", name=f"psnm{l}")
                    for i in range(gn):
                        nc.tensor.transpose(out=ps_nm[:, i, :],
                                            in_=fm_sb[:, i * P:(i + 1) * P],
                                            identity=ident[:HC, :HC])
                    stage = tp.tile([P, GN, HC], b16, name=f"stage{l}")
                    nc.scalar.activation(stage[:, :gn, :], ps_nm[:, :gn, :], AF.Copy,
                                         0.0, 1.0)
                    # tile t holds nodes t*128..t*128+127; pair_local
                    # p_l = t*64 + p//2 -> tok = p_l & 127, k = p_l >> 7;
                    # DRAM (tok, k*128 + (p%2)*64 + f), tok-major rows of PAIRS
                    xp = xl_mine[l][:]
                    for i in range(gn):
                        t = nt0 + i
                        out_ap = bass.AP(
                            tensor=xp.tensor,
                            offset=xp.offset + (64 * (t % 2)) * PAIRS
                            + (t // 2) * P,
                            ap=[[PAIRS, 64], [HC, 2], [1, HC]])
                        nc.sync.dma_start(out=out_ap, in_=stage[:, i, :])
                if "nocoll" in ablate:
                    nc.gpsimd.dma_start(out=xl_pair[l][:PAIRS, :],
                                        in_=xl_mine[l][:])
                else:
                    nc.gpsimd.collective_compute(
                        "AllGather", OP.bypass,
                        replica_groups=[list(range(cfg.CORES))],
                        ins=[xl_mine[l][:]], outs=[xl_pair[l][:]])
                # xr_we blocks from local features (rows 0:NB); rows NB:128 = We
                for b in range(NBLK):
                    ps_b = pp.tile([HC, NB], f32, space="PSUM", name=f"psb{l}")
                    nc.tensor.matmul(out=ps_b[:], lhsT=wsb[f"Wr{l}"][:],
                                     rhs=xin[:, b * NB:(b + 1) * NB],
                                     start=True, stop=True)
                    xr_sb = tp.tile([HC, NB], b16, name=f"xrsb{l}")
                    nc.scalar.activation(xr_sb[:], ps_b[:], AF.Identity,
                                         wsb[f"br{l}"][:], 1.0)
                    ps_t = pp.tile([NB, HC], b16, space="PSUM", name=f"pst{l}")
                    nc.tensor.transpose(out=ps_t[:], in_=xr_sb[:],
                                        identity=ident[:HC, :HC])
                    nc.scalar.activation(xr_we[:NB, b * HC:(b + 1) * HC], ps_t[:],
                                         AF.Copy, 0.0, 1.0)
                nc.sync.dma_start(out=xr_we[NB:, :], in_=dp[f"We_rows{l}"][:])

        # ================= edge stage =================
        def edge_stage(l):
            with ExitStack() as c2:
                ep = c2.enter_context(tc.tile_pool(name=f"ed{l}", bufs=2))
                gp = c2.enter_context(tc.tile_pool(name=f"eg{l}", bufs=3))
                xp = c2.enter_context(tc.tile_pool(name=f"edx{l}", bufs=2, space="PSUM"))
                tp = c2.enter_context(tc.tile_pool(name=f"edo{l}", bufs=2, space="PSUM"))
                mp = c2.enter_context(tc.tile_pool(name=f"edm{l}", bufs=2, space="PSUM"))
                ag = c2.enter_context(tc.tile_pool(name=f"eda{l}", bufs=1, space="PSUM"))
                pt = c2.enter_context(tc.tile_pool(name=f"edq{l}", bufs=1, space="PSUM"))
                # load this layer's pair table into SBUF
                xl_sb = xl_sbs[l]
                if l == 1 and "dblbuf" not in ablate:
                    # WAR fence: act_fm readiness implies every layer-0 gather
                    # has fully drained its read of xl_sb
                    nc.gpsimd.dma_start(out=xl_sb[0:1, 0:1],
                                        in_=act_fm[0:1, 0:1])
                xl_ap = xl_pair[l][:]
                nc.gpsimd.dma_start(
                    out=xl_sb[:].rearrange("p (c x) -> p c x", c=cfg.CORES),
                    in_=bass.AP(tensor=xl_ap.tensor, offset=xl_ap.offset,
                                ap=[[PAIRS, P], [P * PAIRS, cfg.CORES],
                                    [1, PAIRS]]))
                ngr = math.ceil(T_B / 8)
                base, rem = divmod(T_B, ngr)
                groups, t0 = [], 0
                for i in range(ngr):
                    gt = base + (1 if i < rem else 0)
                    groups.append((t0, gt))
                    t0 += gt
                for b in range(NBLK):
                    xlT = ep.tile([P, 1, ET], b16, name=f"xlT{l}")
                    if "nogather" in ablate:
                        nc.sync.dma_start(
                            out=xlT[:],
                            in_=xl_pair[l][:].rearrange(
                                "(a b) p -> a (b p)", a=P)[:, :ET])
                    elif "hbmgather" in ablate:
                        nc.gpsimd.dma_gather(
                            out_ap=xlT[:], in_ap=xl_pair[l][:],
                            idxs_ap=srcpair[:, b * (ET // 16):(b + 1) * (ET // 16)],
                            num_idxs=ET, num_idxs_reg=ET, elem_size=P,
                            transpose=True, single_packet=False)
                    else:
                        nc.gpsimd.dma_gather(
                            out_ap=xlT[:], in_ap=xl_sb[:],
                            idxs_ap=srcpair[:, b * (ET // 16):(b + 1) * (ET // 16)],
                            num_idxs=ET, num_idxs_reg=ET, elem_size=P,
                            transpose=True,
                            single_packet=("sp" in ablate),
                            queue_num=b % nq,
                            sbuf_tokens_per_rank=P,
                            sbuf_free_dim_per_rank=256)
                    if "gatheronly" in ablate:
                        continue
                    agg = ag.tile([P, PAY], f32, space="PSUM", name=f"agg{l}")
                    for gi, (t0, gt) in enumerate(groups):
                        cols = slice(b * T_B + t0, b * T_B + t0 + gt)
                        # edge-major one-hot over dst slots (for aggregation)
                        oh_g = gp.tile([P, 8, P], b16, name=f"oh{l}")
                        nc.vector.tensor_tensor(
                            out=oh_g[:, :gt, :],
                            in0=bcast_f(dstloc[:, cols], P, 2),
                            in1=bcast_f(iotaB[:], gt, 1),
                            op=OP.is_equal)
                        nc.vector.tensor_copy(
                            oh_g[:, :gt, NB:],
                            ea_res[:].rearrange("p (t f) -> p t f",
                                                f=cfg.EDGE_DIM)[:, cols, :])
                        # slot-major one-hot (for xr select): PE transpose
                        ps_o = tp.tile([P, 8, P], b16, space="PSUM", name=f"pso{l}")
                        for k in range(gt):
                            nc.tensor.transpose(out=ps_o[:, k, :],
                                                in_=oh_g[:, k, :],
                                                identity=ident[:])
                        ohT_g = gp.tile([P, 8, P], b16, name=f"ohT{l}")
                        nc.vector.tensor_copy(ohT_g[:, :gt, :], ps_o[:, :gt, :])
                        # edge-major gathered pairs + parity select
                        ps_x = xp.tile([P, 8, P], b16, space="PSUM", name=f"psx{l}")
                        for k in range(gt):
                            nc.tensor.transpose(
                                out=ps_x[:, k, :],
                                in_=xlT[:, 0, (t0 + k) * P:(t0 + k + 1) * P],
                                identity=ident[:])
                        xlsel_t = gp.tile([P, 8, HC + 2], b16, name=f"xls{l}")
                        xlsel_g = xlsel_t[:, :, :HC]
                        nc.scalar.activation(xlsel_t[:, :gt, :HC],
                                             ps_x[:, :gt, :HC], AF.Copy, 0.0, 1.0)
                        xlodd_t = gp.tile([P, 8, HC + 2], b16, name=f"xlo{l}")
                        nc.scalar.activation(xlodd_t[:, :gt, :HC],
                                             ps_x[:, :gt, HC:], AF.Copy, 0.0, 1.0)
                        nc.vector.copy_predicated(
                            xlsel_t[:, :gt, :HC],
                            bcast_f(parity[:, cols], HC, 2),
                            xlodd_t[:, :gt, :HC])
                        if "noedge" in ablate:
                            continue
                        # messages: m = (xr[dst] + eattr@We) + xl[src] in PSUM
                        psm = mp.tile([P, 8, HC], f32, space="PSUM", name=f"psm{l}")
                        for k in range(gt):
                            nc.tensor.matmul(
                                out=psm[:, k, :], lhsT=ohT_g[:, k, :],
                                rhs=xr_we[:, b * HC:(b + 1) * HC],
                                start=True, stop=False)
                            nc.tensor.matmul(
                                out=psm[:, k, :], lhsT=ident[:],
                                rhs=xlsel_t[:, k, :HC],
                                start=False, stop=True)
                        # lrelu(x) = x + 0.8*relu(-x), then * att
                        r2n = gp.tile([P, 8, HC], b16, name=f"r2n{l}")
                        nc.scalar.activation(r2n[:, :gt, :], psm[:, :gt, :],
                                             AF.Relu, 0.0, -1.0)
                        mlr = gp.tile([P, 8, HC], b16, name=f"mlr{l}")
                        nc.vector.scalar_tensor_tensor(
                            out=mlr[:, :gt, :], in0=r2n[:, :gt, :],
                            scalar=0.8, in1=psm[:, :gt, :],
                            op0=OP.mult, op1=OP.add)
                        nc.vector.tensor_tensor(
                            out=mlr[:, :gt, :], in0=mlr[:, :gt, :],
                            in1=bcast_f(wsb[f"att{l}"][:], gt, 1), op=OP.mult)
                        logit = gp.tile([P, 8, H], f32, name=f"lg{l}")
                        nc.vector.tensor_reduce(
                            out=logit[:, :gt, :],
                            in_=mlr[:, :gt, :].rearrange("p t (h c) -> p t h c",
                                                         h=H),
                            axis=mybir.AxisListType.X, op=OP.add)
                        pay = gp.tile([P, 8, PAY], b16, name=f"pay{l}")
                        nc.scalar.activation(pay[:, :gt, HC:], logit[:, :gt, :],
                                             AF.Exp, 0.0, 1.0)
                        nc.vector.tensor_tensor(
                            out=pay[:, :gt, HC:], in0=pay[:, :gt, HC:],
                            in1=bcast_f(valid[:, cols], H, 2),
                            op=OP.mult)
                        nc.vector.tensor_tensor(
                            out=pay[:, :gt, :HC].rearrange(
                                "p t (h c) -> p t h c", h=H),
                            in0=xlsel_t[:, :gt, :HC].rearrange(
                                "p t (h c) -> p t h c", h=H),
                            in1=bcast_f(pay[:, :gt, HC:], C, 3), op=OP.mult)
                        for k in range(gt):
                            nc.tensor.matmul(
                                out=agg[:], lhsT=oh_g[:, k, :], rhs=pay[:, k, :],
                                start=(gi == 0 and k == 0),
                                stop=(gi == ngr - 1 and k == gt - 1))
                    if "noedge" in ablate:
                        continue
                    dena = ep.tile([NB, H], f32, name=f"dena{l}")
                    nc.scalar.activation(dena[:], agg[:NB, HC:], AF.Identity,
                                         c_tiny[:NB, :], 1.0)
                    denr = ep.tile([NB, H], f32, name=f"denr{l}")
                    nc.vector.reciprocal(denr[:], dena[:])
                    xn = ep.tile([NB, HC], f32, name=f"xn{l}")
                    nc.vector.tensor_tensor(
                        out=xn[:].rearrange("n (h c) -> n h c", h=H),
                        in0=agg[:NB, :HC].rearrange("n (h c) -> n h c", h=H),
                        in1=bcast_f(denr[:], C, 2), op=OP.mult)
                    ps_t = pt.tile([HC, NB], f32, space="PSUM", name=f"epst{l}")
                    nc.tensor.transpose(out=ps_t[:], in_=xn[:],
                                        identity=identF[:NB, :NB])
                    nc.scalar.activation(act_pre[:, b * NB:(b + 1) * NB], ps_t[:],
                                         AF.Copy, 0.0, 1.0)

        # ================= BN + ELU (feature-major, per-partition channels) ====
        def bn_elu(x_sb, F, ncols, nlp, g_ap, be_ap, tag, ach=2048, bbufs=2):
            """x_sb [F, nlp] f32 tile; stats over first ncols cols; applies
            ELU(BN(x)) in place. Uses AllReduce for global stats."""
            with ExitStack() as c2:
                bp = c2.enter_context(tc.tile_pool(name=f"bn{tag}", bufs=bbufs))
                ssum = bp.tile([F, 2], f32, name=f"ssum{tag}")
                nc.vector.tensor_reduce(out=ssum[:, 0:1], in_=x_sb[:, :ncols],
                                        axis=mybir.AxisListType.X, op=OP.add)
                BCH = 1024
                nbch = math.ceil(ncols / BCH)
                sq_parts = bp.tile([F, nbch], f32, name=f"sqp{tag}")
                for bi in range(nbch):
                    c0 = bi * BCH
                    cw = min(BCH, ncols - c0)
                    sq = bp.tile([F, BCH], f32, name=f"sq{tag}")
                    nc.scalar.activation(sq[:, :cw], x_sb[:, c0:c0 + cw],
                                         AF.Square, 0.0, 1.0,
                                         accum_out=sq_parts[:, bi:bi + 1])
                nc.vector.tensor_reduce(out=ssum[:, 1:2], in_=sq_parts[:],
                                        axis=mybir.AxisListType.X, op=OP.add)
                nc.sync.dma_start(out=st_in[:F, :], in_=ssum[:])
                if "nocoll" in ablate:
                    nc.gpsimd.dma_start(out=st_out[:F, :], in_=st_in[:F, :])
                else:
                    nc.gpsimd.collective_compute(
                        "AllReduce", OP.add,
                        replica_groups=[list(range(cfg.CORES))],
                        ins=[st_in[:F, :]], outs=[st_out[:F, :]])
                gs = bp.tile([F, 2], f32, name=f"gs{tag}")
                nc.sync.dma_start(out=gs[:], in_=st_out[:F, :])
                mean = bp.tile([F, 1], f32, name=f"mean{tag}")
                nc.scalar.activation(mean[:], gs[:, 0:1], AF.Copy, 0.0, 1.0 / cfg.N)
                msq = bp.tile([F, 1], f32, name=f"msq{tag}")
                nc.scalar.activation(msq[:], gs[:, 1:2], AF.Copy, 0.0, 1.0 / cfg.N)
                # var = msq - mean^2 (biased)
                m2 = bp.tile([F, 1], f32, name=f"m2{tag}")
                nc.vector.tensor_tensor(out=m2[:], in0=mean[:], in1=mean[:],
                                        op=OP.mult)
                var = bp.tile([F, 1], f32, name=f"var{tag}")
                nc.vector.tensor_tensor(out=var[:], in0=msq[:], in1=m2[:],
                                        op=OP.subtract)
                vare = bp.tile([F, 1], f32, name=f"vare{tag}")
                nc.scalar.activation(vare[:], var[:], AF.Identity, c_eps[:F, :], 1.0)
                vrec = bp.tile([F, 1], f32, name=f"vrec{tag}")
                nc.vector.reciprocal(vrec[:], vare[:])
                rstd = bp.tile([F, 1], f32, name=f"rstd{tag}")
                nc.scalar.activation(rstd[:], vrec[:], AF.Sqrt, 0.0, 1.0)
                scl = bp.tile([F, 1], f32, name=f"scl{tag}")
                nc.vector.tensor_tensor(out=scl[:], in0=g_ap, in1=rstd[:], op=OP.mult)
                sht = bp.tile([F, 1], f32, name=f"sht{tag}")
                nc.vector.tensor_tensor(out=sht[:], in0=mean[:], in1=scl[:], op=OP.mult)
                nc.vector.tensor_tensor(out=sht[:], in0=be_ap, in1=sht[:], op=OP.subtract)
                ACH = ach
                nach = math.ceil(nlp / ACH)
                for ai in range(nach):
                    c0 = ai * ACH
                    cw = min(ACH, nlp - c0)
                    y = bp.tile([F, ACH], f32, name=f"y{tag}")
                    nc.vector.scalar_tensor_tensor(
                        out=y[:, :cw], in0=x_sb[:, c0:c0 + cw], scalar=scl[:],
                        in1=bass.AP(tensor=sht.tensor, offset=sht[:].offset,
                                    ap=[sht[:].ap[0], [0, cw]]),
                        op0=OP.mult, op1=OP.add)
                    r = bp.tile([F, ACH], f32, name=f"r{tag}")
                    nc.scalar.activation(r[:, :cw], y[:, :cw], AF.Relu, 0.0, 1.0)
                    ng = bp.tile([F, ACH], f32, name=f"ng{tag}")
                    nc.vector.tensor_tensor(out=ng[:, :cw], in0=y[:, :cw],
                                            in1=r[:, :cw], op=OP.subtract)
                    eg = bp.tile([F, ACH], f32, name=f"eg{tag}")
                    nc.scalar.activation(eg[:, :cw], ng[:, :cw], AF.Exp, 0.0, 1.0)
                    nc.vector.scalar_tensor_tensor(
                        out=x_sb[:, c0:c0 + cw], in0=eg[:, :cw], scalar=-1.0,
                        in1=r[:, :cw], op0=OP.add, op1=OP.add)

        # ================= main program =================
        nc.vector.memset(act_pre[:], 0.01)
        if ablate & {"noedge", "noedgestage", "minimal", "empty"}:
            nc.vector.tensor_copy(act_fm[:], act_pre[:])
        for l in range(2):
            if "empty" in ablate:
                break
            if "minimal" not in ablate:
                build_tables(l)
                if "noedgestage" not in ablate:
                    edge_stage(l)
            if dbg_p is not None and l == int("debug2" in ablate) \
                    and "debug3" not in ablate:
                nc.sync.dma_start(out=dbg_p[:], in_=act_pre[:])
            bn_elu(act_pre, HC, NL, NLP, wsb[f"g{l}"][:], wsb[f"be{l}"][:], f"l{l}",
                   ach=1024)
            nc.vector.tensor_copy(act_fm[:], act_pre[:])
            if dbg_p is not None and l == 1 and "debug3" in ablate:
                nc.sync.dma_start(out=dbg_p[:], in_=act_fm[:])

        # ---- head: x @ W1 -> BN -> ELU -> @ W2 -> 5*tanh
        with ExitStack() as c2:
            hp = c2.enter_context(tc.tile_pool(name="head", bufs=1))
            hpp = c2.enter_context(tc.tile_pool(name="headp", bufs=2, space="PSUM"))
            x3 = hp.tile([cfg.HS2, NLP], b16, name="x3")
            CH = 512
            nch = math.ceil(NLP / CH)
            for ci in range(nch):
                c0 = ci * CH
                cw = min(CH, NLP - c0)
                psh = hpp.tile([cfg.HS2, CH], f32, space="PSUM", name="psh")
                nc.tensor.matmul(out=psh[:, :cw], lhsT=wsb["W1"][:],
                                 rhs=act_fm[:, c0:c0 + cw], start=True, stop=True)
                nc.scalar.activation(x3[:, c0:c0 + cw], psh[:, :cw], AF.Copy,
                                     0.0, 1.0)
            bn_elu(x3, cfg.HS2, NL, NLP, wsb["gf"][:], wsb["bf"][:], "hd",
                   ach=1024, bbufs=1)
            for ci in range(nch):
                c0 = ci * CH
                if c0 >= NL:
                    break
                cw = min(CH, NLP - c0)
                cv = min(cw, NL - c0)
                pso = hpp.tile([1, CH], f32, space="PSUM", name="pso")
                nc.tensor.matmul(out=pso[:, :cw], lhsT=wsb["W2"][:],
                                 rhs=x3[:, c0:c0 + cw], start=True, stop=True)
                th = hp.tile([1, CH], f32, name="th", bufs=2)
                nc.scalar.activation(th[:, :cw], pso[:, :cw], AF.Tanh,
                                     wsb["b2"][:], 1.0)
                nc.vector.tensor_scalar(out=th[:, :cw], in0=th[:, :cw],
                                        scalar1=5.0, scalar2=None, op0=OP.mult)
                nc.sync.dma_start(out=out_p[:, c0:c0 + cv], in_=th[:, :cv])

    nc.compile()
    return nc


# ---------------------------------------------------------------- entry

_CACHE = {}


def _get_built(cfg: Cfg, T_B: int):
    key = (cfg.N, cfg.E, T_B)
    if key not in _CACHE:
        _CACHE[key] = build(cfg, T_B)
    return _CACHE[key]


def run(cfg: Cfg, inputs: dict):
    w = prep_weights(cfg, inputs)
    in_maps, T_B = prep(cfg, inputs["data_x"], inputs["data_edge_index"],
                        inputs["data_edge_attr"], w)
    nc = _get_built(cfg, T_B)
    res = run_bass_kernel_spmd(nc, in_maps, core_ids=list(range(cfg.CORES)))
    out = np.concatenate([np.asarray(res.results[c]["out"]).reshape(-1)
                          for c in range(cfg.CORES)])
    return out.reshape(cfg.N, 1).astype(np.float32)


def kernel(**inputs):
    return run(FULL, inputs)


# ---------------------------------------------------------------- timing

def time_kernel(inputs, iters=20):
    """Build the jitted 8-core executable once, run it `iters` times with
    device-resident inputs, return average per-execution wall time in ns."""
    import time
    import jax
    from jax.experimental.shard_map import shard_map
    from jax.sharding import Mesh, PartitionSpec, NamedSharding
    from concourse import bass2jax, mybir as _mb

    cfg = FULL
    w = prep_weights(cfg, inputs)
    in_maps, T_B = prep(cfg, inputs["data_x"], inputs["data_edge_index"],
                        inputs["data_edge_attr"], w)
    nc = _get_built(cfg, T_B)
    bass2jax.install_neuronx_cc_hook()
    n_cores = cfg.CORES
    partition_name = nc.partition_id_tensor.name if nc.partition_id_tensor else None
    in_names, out_names, out_avals, zero_outs = [], [], [], []
    for alloc in nc.m.functions[0].allocations:
        if not isinstance(alloc, _mb.MemoryLocationSet):
            continue
        name = alloc.memorylocations[0].name
        if alloc.kind == "ExternalInput":
            if name != partition_name:
                in_names.append(name)
        elif alloc.kind == "ExternalOutput":
            out_names.append(name)
            shape = tuple(alloc.tensor_shape)
            dtype = _mb.dt.np(alloc.dtype)
            out_avals.append(jax.core.ShapedArray(shape, dtype))
            zero_outs.append(np.zeros(shape, dtype))
    n_params = len(in_names)
    all_in = list(in_names) + list(out_names)
    if partition_name is not None:
        all_in.append(partition_name)

    def _body(*args):
        operands = list(args)
        if partition_name is not None:
            operands.append(bass2jax.partition_id_tensor())
        outs = bass2jax._bass_exec_p.bind(
            *operands,
            out_avals=tuple(out_avals),
            in_names=tuple(all_in),
            out_names=tuple(out_names),
            lowering_input_output_aliases=(),
            sim_require_finite=True,
            sim_require_nnan=True,
            nc=nc,
        )
        return tuple(outs)

    devices = jax.devices()[:n_cores]
    mesh = Mesh(np.asarray(devices), ("core",))
    n_outs = len(out_names)
    in_specs = (PartitionSpec("core"),) * (n_params + n_outs)
    out_specs = (PartitionSpec("core"),) * n_outs
    sharded = jax.jit(
        shard_map(_body, mesh=mesh, in_specs=in_specs, out_specs=out_specs,
                  check_rep=False),
        keep_unused=True)
    sh = NamedSharding(mesh, PartitionSpec("core"))
    concat_in = [
        jax.device_put(
            np.concatenate([np.asarray(in_maps[c][nm]) for c in range(n_cores)],
                           axis=0), sh)
        for nm in in_names]
    concat_zeros = [
        jax.device_put(np.zeros((n_cores * z.shape[0], *z.shape[1:]), z.dtype), sh)
        for z in zero_outs]
    outs = sharded(*concat_in, *concat_zeros)  # warm-up (compiles)
    jax.block_until_ready(outs)
    for _ in range(15):  # steady-state warm-up
        outs = sharded(*concat_in, *concat_zeros)
    jax.block_until_ready(outs)
    iters = max(iters, 300)
    t0 = time.perf_counter()
    for _ in range(iters):
        outs = sharded(*concat_in, *concat_zeros)
    jax.block_until_ready(outs)
    t1 = time.perf_counter()
    return (t1 - t0) / iters * 1e9
